# revision 7
# baseline (speedup 1.0000x reference)
"""AttentionRouter Trainium2 kernel.

Computes, for packed tokens x [T=32768, H=8, D=128] with B=8 ragged segments
(cu_seq_len [9]), the per-segment mean-pooled features -> tiny MLP router ->
binary mask z [B, H, 1].

Strategy (8 NeuronCores, data-parallel over tokens):
  - Each core owns 4096 tokens (16 MiB of x). Viewed as [4096, 1024].
  - Segment membership masks are built on-device from cu_seq_len via
    compare ops on a host-supplied token-index iota.
  - Partial segment sums (summed over tokens AND heads) + token counts are
    computed with TensorE mask-matmuls accumulating in PSUM.
  - A tiny (8x129 f32) AllReduce combines partials across cores.
  - Every core then (redundantly) runs the 5-layer MLP on the pooled means
    and emits the final mask; the host takes core 0's output.
"""

import sys

if "/opt/trn_rl_repo" not in sys.path:
    sys.path.insert(0, "/opt/trn_rl_repo")

import numpy as np

import concourse.bass as bass
import concourse.bacc as bacc
import concourse.tile as tile
from concourse import mybir
from concourse.bass_utils import run_bass_kernel_spmd

N_CORES = 8
T, B, H, D = 32768, 8, 8, 128
E = H * D                      # 1024 features per token (heads folded in)
TOK = T // N_CORES             # 4096 tokens per core
NPART = 128
TPB = TOK // NPART             # 32 token-blocks (matmul contraction tiles)
NCHUNK = 8                     # x DMA chunks per core
BPC = TPB // NCHUNK            # 4 token-blocks per DMA chunk

F32 = mybir.dt.float32


def _mlp_dense(nc, wp, pp_mlp, sp, ones_row, a_in, w_sb, b_sb, K, M, act):
    """out[M, 8] = act(W.T @ a_in + b). a_in: [128, kch*8] sbuf, chunk k at
    cols [k*8,(k+1)*8). w_sb: [128, kch, M]. b_sb: [1, M]. Returns [128, mch*8]."""
    kch = K // 128
    mch = (M + 127) // 128
    a_out = sp.tile([128, mch * 8], F32, tag="act")
    for m in range(mch):
        mm = min(128, M - m * 128)
        ps = pp_mlp.tile([128, 8], F32, tag="mlp_ps")
        for k in range(kch):
            nc.tensor.matmul(
                ps[0:mm, :],
                w_sb[:, k, m * 128 : m * 128 + mm],
                a_in[:, k * 8 : (k + 1) * 8],
                start=(k == 0),
                stop=False,
            )
        nc.tensor.matmul(
            ps[0:mm, :],
            b_sb[0:1, m * 128 : m * 128 + mm],
            ones_row[:],
            start=False,
            stop=True,
        )
        if act:
            # silu(x) = x * sigmoid(x)  (Silu is not implemented in CoreSim)
            sg = sp.tile([128, 8], F32, tag="mlp_sig")
            nc.scalar.activation(
                sg[0:mm, :], ps[0:mm, :], mybir.ActivationFunctionType.Sigmoid
            )
            nc.vector.tensor_tensor(
                a_out[0:mm, m * 8 : (m + 1) * 8], ps[0:mm, :], sg[0:mm, :],
                op=mybir.AluOpType.mult,
            )
        else:
            nc.scalar.copy(a_out[0:mm, m * 8 : (m + 1) * 8], ps[0:mm, :])
    return a_out


def _build_kernel_body(nc, tc, d):
    """d: dict of DRAM tensor handles."""
    with (
        tc.tile_pool(name="xp", bufs=4) as xp,
        tc.tile_pool(name="wp", bufs=1) as wp,
        tc.tile_pool(name="sp", bufs=1) as sp,
        tc.tile_pool(name="spa", bufs=2) as spa,
        tc.tile_pool(name="pp", bufs=1, space="PSUM") as pp,
        tc.tile_pool(name="ppm", bufs=3, space="PSUM") as ppm,
        tc.tile_pool(name="dp", bufs=1, space="DRAM") as dp,
    ):
        # ---- small constants / metadata ----
        cu_sb = sp.tile([128, B + 1], F32)
        nc.sync.dma_start(cu_sb[:], d["cu"].ap())
        tidx = sp.tile([128, TPB], F32)
        nc.sync.dma_start(tidx[:], d["tidx"].ap())
        ident = sp.tile([8, 8], F32)
        nc.sync.dma_start(ident[:], d["ident"].ap())

        ones_col = sp.tile([128, 1], F32)
        nc.vector.memset(ones_col[:], 1.0)
        ones_row = sp.tile([1, 8], F32)
        nc.vector.memset(ones_row[:], 1.0)
        ones8 = sp.tile([8, 8], F32)
        nc.vector.memset(ones8[:], 1.0)

        # ---- segment membership masks from cu_seq_len ----
        # ge[p, j, n] = (token_idx[p, n] >= cu[j]);  mask = ge[:,0:8]-ge[:,1:9]
        ge = sp.tile([128, B + 1, TPB], F32)
        for j in range(B + 1):
            nc.vector.tensor_scalar(
                ge[:, j, :],
                tidx[:],
                cu_sb[:, j : j + 1],
                None,
                op0=mybir.AluOpType.is_ge,
            )
        # mask[p, b, n]: token of (p, n) belongs to segment b
        mask = sp.tile([128, B, TPB], F32)
        nc.vector.tensor_tensor(
            mask[:], ge[:, 0:B, :], ge[:, 1 : B + 1, :], op=mybir.AluOpType.subtract
        )

        # ---- MLP weights to SBUF (overlaps with phase 1 below) ----
        w1_sb = wp.tile([128, 1, 8 * D], F32)   # [128, 1024]
        nc.sync.dma_start(w1_sb[:], d["w1"].ap().rearrange("(k p) m -> p k m", p=128))
        w2_sb = wp.tile([128, 8, 2 * D], F32)   # W2 [1024, 256]
        nc.sync.dma_start(w2_sb[:], d["w2"].ap().rearrange("(k p) m -> p k m", p=128))
        w3_sb = wp.tile([128, 2, 4 * D], F32)   # W3 [256, 512]
        nc.sync.dma_start(w3_sb[:], d["w3"].ap().rearrange("(k p) m -> p k m", p=128))
        w4_sb = wp.tile([128, 4, D], F32)       # W4 [512, 128]
        nc.sync.dma_start(w4_sb[:], d["w4"].ap().rearrange("(k p) m -> p k m", p=128))
        w5_sb = wp.tile([128, 1, 2], F32)       # W5 [128, 2]
        nc.sync.dma_start(w5_sb[:], d["w5"].ap().rearrange("(k p) m -> p k m", p=128))
        b_sbs = {}
        for name, n in (("b1", 8 * D), ("b2", 2 * D), ("b3", 4 * D), ("b4", D), ("b5", 2)):
            b_sbs[name] = wp.tile([1, n], F32, tag=name, name=f"{name}_sb")
            nc.sync.dma_start(b_sbs[name][:], d[name].ap())

        # ---- phase 1: masked segment sums over this core's tokens ----
        # x viewed [128, TPB, E]: partition p, block n holds token p*TPB + n.
        xv = d["x"].ap().rearrange("(p n) e -> p n e", p=128)
        ps0 = pp.tile([B, 512], F32)    # segment sums, features 0:512
        ps1 = pp.tile([B, 512], F32)    # segment sums, features 512:1024
        psc = pp.tile([B, 1], F32)      # segment token counts
        for c in range(NCHUNK):
            xt = xp.tile([128, BPC, E], F32, tag="xt")
            nc.sync.dma_start(xt[:], xv[:, c * BPC : (c + 1) * BPC, :])
            for k in range(BPC):
                n = c * BPC + k
                first, last = (n == 0), (n == TPB - 1)
                lhsT = mask[:, :, n]
                nc.tensor.matmul(ps0[:], lhsT, xt[:, k, 0:512], start=first, stop=last)
                nc.tensor.matmul(ps1[:], lhsT, xt[:, k, 512:E], start=first, stop=last)
                nc.tensor.matmul(psc[:], lhsT, ones_col[:], start=first, stop=last)

        # ---- head-sum: [B, 1024] -> [B, 128]; pack counts alongside ----
        # (tensor_tensor may read at most one input from PSUM)
        sb1 = sp.tile([B, 512], F32)
        nc.scalar.copy(sb1[:], ps1[:])
        s512 = sp.tile([B, 512], F32)
        nc.vector.tensor_tensor(s512[:], ps0[:], sb1[:], op=mybir.AluOpType.add)
        pre = sp.tile([B, D + 1], F32)
        s256 = sp.tile([B, 256], F32)
        nc.vector.tensor_tensor(
            s256[:], s512[:, 0:256], s512[:, 256:512], op=mybir.AluOpType.add
        )
        nc.vector.tensor_tensor(
            pre[:, 0:D], s256[:, 0:128], s256[:, 128:256], op=mybir.AluOpType.add
        )
        nc.vector.tensor_copy(pre[:, D : D + 1], psc[:])

        # ---- AllReduce partial sums + counts across the 8 cores ----
        arin = dp.tile([B, D + 1], F32)
        arout = dp.tile([B, D + 1], F32)
        nc.sync.dma_start(arin[:], pre[:])
        nc.gpsimd.collective_compute(
            "AllReduce",
            mybir.AluOpType.add,
            replica_groups=[list(range(N_CORES))],
            ins=[arin.opt()],
            outs=[arout.opt()],
        )
        post = sp.tile([B, D + 1], F32)
        nc.sync.dma_start(post[:], arout[:])

        # ---- pooled mean: sums / (H * max(count, 1)) ----
        denom = sp.tile([B, 1], F32)
        nc.vector.tensor_scalar(
            denom[:], post[:, D : D + 1], 1.0, float(H),
            op0=mybir.AluOpType.max, op1=mybir.AluOpType.mult,
        )
        recip = sp.tile([B, 1], F32)
        nc.vector.reciprocal(recip[:], denom[:])
        pm = sp.tile([B, D], F32)
        nc.vector.tensor_scalar(
            pm[:], post[:, 0:D], recip[:], None, op0=mybir.AluOpType.mult
        )

        # ---- transpose pooled mean -> a0 [128, 8] ----
        pmt = ppm.tile([D, B], F32, tag="mlp_ps")
        nc.tensor.transpose(pmt[:], pm[:], ident[:])
        a0 = sp.tile([D, B], F32)
        nc.scalar.copy(a0[:], pmt[:])

        # ---- MLP (activations kept transposed: [feature, batch]) ----
        a1 = _mlp_dense(nc, wp, ppm, spa, ones_row, a0, w1_sb, b_sbs["b1"], D, 8 * D, True)
        a2 = _mlp_dense(nc, wp, ppm, spa, ones_row, a1, w2_sb, b_sbs["b2"], 8 * D, 2 * D, False)
        a3 = _mlp_dense(nc, wp, ppm, spa, ones_row, a2, w3_sb, b_sbs["b3"], 2 * D, 4 * D, True)
        a4 = _mlp_dense(nc, wp, ppm, spa, ones_row, a3, w4_sb, b_sbs["b4"], 4 * D, D, True)
        a5 = _mlp_dense(nc, wp, ppm, spa, ones_row, a4, w5_sb, b_sbs["b5"], D, 2, False)

        # ---- logits [2, 8] -> z[b] = (logit1 > logit0) -> out [8, 8] ----
        lgt = ppm.tile([B, 2], F32, tag="mlp_ps")
        nc.tensor.transpose(lgt[:], a5[0:2, 0:8], ident[0:2, 0:2])
        lg = sp.tile([B, 2], F32)
        nc.scalar.copy(lg[:], lgt[:])
        z = sp.tile([B, 1], F32)
        nc.vector.tensor_tensor(z[:], lg[:, 1:2], lg[:, 0:1], op=mybir.AluOpType.is_gt)
        zb = sp.tile([B, H], F32)
        nc.vector.tensor_scalar(zb[:], ones8[:], z[:], None, op0=mybir.AluOpType.mult)
        nc.sync.dma_start(d["out"].ap(), zb[:])


def build():
    nc = bacc.Bacc("TRN2", target_bir_lowering=False, debug=False, num_devices=N_CORES)
    d = {}
    d["x"] = nc.dram_tensor("x", [TOK, E], F32, kind="ExternalInput")
    d["tidx"] = nc.dram_tensor("tidx", [NPART, TPB], F32, kind="ExternalInput")
    d["cu"] = nc.dram_tensor("cu", [NPART, B + 1], F32, kind="ExternalInput")
    d["ident"] = nc.dram_tensor("ident", [8, 8], F32, kind="ExternalInput")
    d["w1"] = nc.dram_tensor("w1", [D, 8 * D], F32, kind="ExternalInput")
    d["b1"] = nc.dram_tensor("b1", [1, 8 * D], F32, kind="ExternalInput")
    d["w2"] = nc.dram_tensor("w2", [8 * D, 2 * D], F32, kind="ExternalInput")
    d["b2"] = nc.dram_tensor("b2", [1, 2 * D], F32, kind="ExternalInput")
    d["w3"] = nc.dram_tensor("w3", [2 * D, 4 * D], F32, kind="ExternalInput")
    d["b3"] = nc.dram_tensor("b3", [1, 4 * D], F32, kind="ExternalInput")
    d["w4"] = nc.dram_tensor("w4", [4 * D, D], F32, kind="ExternalInput")
    d["b4"] = nc.dram_tensor("b4", [1, D], F32, kind="ExternalInput")
    d["w5"] = nc.dram_tensor("w5", [D, 2], F32, kind="ExternalInput")
    d["b5"] = nc.dram_tensor("b5", [1, 2], F32, kind="ExternalInput")
    d["out"] = nc.dram_tensor("out", [B, H], F32, kind="ExternalOutput")
    with tile.TileContext(nc) as tc:
        _build_kernel_body(nc, tc, d)
    nc.compile()
    return nc


def make_in_maps(x, cu_seq_len, w1, b1, w2, b2, w3, b3, w4, b4, w5, b5):
    x = np.ascontiguousarray(np.asarray(x, dtype=np.float32)).reshape(T, E)
    cu_f = np.asarray(cu_seq_len, dtype=np.float32)
    cu_rep = np.ascontiguousarray(np.broadcast_to(cu_f, (NPART, B + 1)))
    ident = np.eye(8, dtype=np.float32)
    common = {
        "cu": cu_rep,
        "ident": ident,
        "w1": np.asarray(w1, np.float32), "b1": np.asarray(b1, np.float32).reshape(1, -1),
        "w2": np.asarray(w2, np.float32), "b2": np.asarray(b2, np.float32).reshape(1, -1),
        "w3": np.asarray(w3, np.float32), "b3": np.asarray(b3, np.float32).reshape(1, -1),
        "w4": np.asarray(w4, np.float32), "b4": np.asarray(b4, np.float32).reshape(1, -1),
        "w5": np.asarray(w5, np.float32), "b5": np.asarray(b5, np.float32).reshape(1, -1),
    }
    in_maps = []
    for c in range(N_CORES):
        tidx = (c * TOK + np.arange(TOK, dtype=np.float32)).reshape(NPART, TPB)
        in_maps.append({"x": x[c * TOK : (c + 1) * TOK], "tidx": tidx, **common})
    return in_maps


_NC_CACHE = {}


def _get_nc():
    if "nc" not in _NC_CACHE:
        _NC_CACHE["nc"] = build()
    return _NC_CACHE["nc"]


def kernel(**inputs):
    nc = _get_nc()
    in_maps = make_in_maps(**inputs)
    res = run_bass_kernel_spmd(nc, in_maps, core_ids=list(range(N_CORES)))
    out = np.asarray(res.results[0]["out"], dtype=np.float32)
    return out.reshape(B, H, 1)


# revision 8
# speedup vs baseline: 1.1621x; 1.1621x over previous
"""AttentionRouter Trainium2 kernel.

Computes, for packed tokens x [T=32768, H=8, D=128] with B=8 ragged segments
(cu_seq_len [9]), the per-segment mean-pooled features -> tiny MLP router ->
binary mask z [B, H, 1].

Strategy (8 NeuronCores, data-parallel over tokens):
  - Each core owns 4096 tokens (16 MiB of x, read as f32; cast to bf16
    in-flight by gpsimd cast-DMA — PSUM accumulation stays f32).
  - Segment membership masks are built on-device from cu_seq_len via
    compare ops on a host-supplied token-index iota.
  - Partial segment sums (over tokens AND heads) via TensorE mask-matmuls.
  - A tiny (8x128 f32) AllReduce combines partials across cores; segment
    counts come from cu_seq_len directly (replicated on every core).
  - Every core then (redundantly) runs the 5-layer MLP on the pooled means
    and emits the final mask; the host takes core 0's output.
"""

import sys

if "/opt/trn_rl_repo" not in sys.path:
    sys.path.insert(0, "/opt/trn_rl_repo")

import numpy as np

import concourse.bass as bass
import concourse.bacc as bacc
import concourse.tile as tile
from concourse import mybir
from concourse.bass_utils import run_bass_kernel_spmd

N_CORES = 8
T, B, H, D = 32768, 8, 8, 128
E = H * D                      # 1024 features per token (heads folded in)
TOK = T // N_CORES             # 4096 tokens per core
NPART = 128
TPB = TOK // NPART             # 32 token-blocks (matmul contraction tiles)
NCHUNK = 8                     # x DMA chunks per core
BPC = TPB // NCHUNK            # 4 token-blocks per DMA chunk

F32 = mybir.dt.float32
BF16 = mybir.dt.bfloat16


def _mlp_dense(nc, pp_mlp, sp, ones_row, a_in, w_sb, b_sb, K, M, act):
    """out[M, 8] = act(W.T @ a_in + b), activations transposed [feat, batch].
    a_in: [128, kch*8] bf16, chunk k at cols [k*8,(k+1)*8). w_sb: [128, kch, M]
    bf16. b_sb: [1, M] bf16. Returns bf16 [128, mch*8]."""
    kch = K // 128
    mch = (M + 127) // 128
    a_out = sp.tile([128, mch * 8], BF16, tag="act")
    for m in range(mch):
        mm = min(128, M - m * 128)
        ps = pp_mlp.tile([128, 8], F32, tag="mlp_ps")
        for k in range(kch):
            nc.tensor.matmul(
                ps[0:mm, :],
                w_sb[:, k, m * 128 : m * 128 + mm],
                a_in[:, k * 8 : (k + 1) * 8],
                start=(k == 0),
                stop=False,
            )
        nc.tensor.matmul(
            ps[0:mm, :],
            b_sb[0:1, m * 128 : m * 128 + mm],
            ones_row[:],
            start=False,
            stop=True,
        )
        if act:
            # silu(x) = x * sigmoid(x); sigmoid on ACT (table stays loaded),
            # copies/mults on DVE to avoid ACT function-table swaps
            sg = sp.tile([128, 8], BF16, tag="mlp_sig")
            nc.scalar.activation(
                sg[0:mm, :], ps[0:mm, :], mybir.ActivationFunctionType.Sigmoid
            )
            pc = sp.tile([128, 8], BF16, tag="mlp_cp")
            nc.vector.tensor_copy(pc[0:mm, :], ps[0:mm, :])
            nc.vector.tensor_tensor(
                a_out[0:mm, m * 8 : (m + 1) * 8], pc[0:mm, :], sg[0:mm, :],
                op=mybir.AluOpType.mult,
            )
        else:
            nc.vector.tensor_copy(a_out[0:mm, m * 8 : (m + 1) * 8], ps[0:mm, :])
    return a_out


def _build_kernel_body(nc, tc, d):
    """d: dict of DRAM tensor handles."""
    with (
        tc.tile_pool(name="xp", bufs=6) as xp,
        tc.tile_pool(name="wp", bufs=1) as wp,
        tc.tile_pool(name="sp", bufs=1) as sp,
        tc.tile_pool(name="spa", bufs=2) as spa,
        tc.tile_pool(name="pp", bufs=1, space="PSUM") as pp,
        tc.tile_pool(name="ppm", bufs=3, space="PSUM") as ppm,
        tc.tile_pool(name="dp", bufs=1, space="DRAM") as dp,
    ):
        # ---- small constants / metadata ----
        cu_sb = sp.tile([128, B + 1], F32)
        nc.sync.dma_start(cu_sb[:], d["cu"].ap())
        tidx = sp.tile([128, TPB], F32)
        nc.sync.dma_start(tidx[:], d["tidx"].ap())
        ident = sp.tile([8, 8], F32)
        nc.sync.dma_start(ident[:], d["ident"].ap())

        ones_row = sp.tile([1, 8], BF16)
        nc.vector.memset(ones_row[:], 1.0)
        ones8 = sp.tile([8, 8], F32)
        nc.vector.memset(ones8[:], 1.0)

        # ---- segment membership masks from cu_seq_len ----
        # ge[p, j, n] = (token_idx[p, n] >= cu[j]);  mask = ge[:,0:8]-ge[:,1:9]
        ge = sp.tile([128, B + 1, TPB], F32)
        for j in range(B + 1):
            nc.vector.tensor_scalar(
                ge[:, j, :],
                tidx[:],
                cu_sb[:, j : j + 1],
                None,
                op0=mybir.AluOpType.is_ge,
            )
        # mask[p, b, n]: token of (p, n) belongs to segment b (0/1, exact bf16)
        mask = sp.tile([128, B, TPB], BF16)
        nc.vector.tensor_tensor(
            mask[:], ge[:, 0:B, :], ge[:, 1 : B + 1, :], op=mybir.AluOpType.subtract
        )

        # ---- segment counts from cu (replicated; no collective needed) ----
        counts_row = sp.tile([1, B], F32)
        nc.vector.tensor_tensor(
            counts_row[:], cu_sb[0:1, 1 : B + 1], cu_sb[0:1, 0:B],
            op=mybir.AluOpType.subtract,
        )
        cnt_ps = ppm.tile([B, 1], F32, tag="mlp_ps")
        nc.tensor.matmul(  # transpose [1,B] -> [B,1] via K=1 matmul with ones
            cnt_ps[:], counts_row[:], ident[0:1, 0:1], start=True, stop=True
        )
        # denom = H * max(count, 1)
        denom = sp.tile([B, 1], F32)
        nc.vector.tensor_scalar(
            denom[:], cnt_ps[:], 1.0, float(H),
            op0=mybir.AluOpType.max, op1=mybir.AluOpType.mult,
        )
        recip = sp.tile([B, 1], F32)
        nc.vector.reciprocal(recip[:], denom[:])

        # ---- MLP weights to SBUF as bf16 (gpsimd cast-DMA; overlaps phase 1) ----
        w1_sb = wp.tile([128, 1, 8 * D], BF16)   # W1 [128, 1024]
        nc.gpsimd.dma_start(w1_sb[:], d["w1"].ap().rearrange("(k p) m -> p k m", p=128))
        w2_sb = wp.tile([128, 8, 2 * D], BF16)   # W2 [1024, 256]
        nc.gpsimd.dma_start(w2_sb[:], d["w2"].ap().rearrange("(k p) m -> p k m", p=128))
        w3_sb = wp.tile([128, 2, 4 * D], BF16)   # W3 [256, 512]
        nc.gpsimd.dma_start(w3_sb[:], d["w3"].ap().rearrange("(k p) m -> p k m", p=128))
        w4_sb = wp.tile([128, 4, D], BF16)       # W4 [512, 128]
        nc.gpsimd.dma_start(w4_sb[:], d["w4"].ap().rearrange("(k p) m -> p k m", p=128))
        w5_sb = wp.tile([128, 1, 2], BF16)       # W5 [128, 2]
        nc.gpsimd.dma_start(w5_sb[:], d["w5"].ap().rearrange("(k p) m -> p k m", p=128))
        b_sbs = {}
        for name, n in (("b1", 8 * D), ("b2", 2 * D), ("b3", 4 * D), ("b4", D), ("b5", 2)):
            b_sbs[name] = wp.tile([1, n], BF16, tag=name, name=f"{name}_sb")
            nc.gpsimd.dma_start(b_sbs[name][:], d[name].ap())

        # ---- phase 1: masked segment sums over this core's tokens ----
        # x viewed [128, TPB, E]: partition p, block n holds token p*TPB + n.
        # gpsimd DMA reads f32 from HBM, casts to bf16 on the way into SBUF.
        xv = d["x"].ap().rearrange("(p n) e -> p n e", p=128)
        ps0 = pp.tile([B, 512], F32)    # segment sums, features 0:512
        ps1 = pp.tile([B, 512], F32)    # segment sums, features 512:1024
        for c in range(NCHUNK):
            xt = xp.tile([128, BPC, E], BF16, tag="xt")
            nc.gpsimd.dma_start(xt[:], xv[:, c * BPC : (c + 1) * BPC, :])
            for k in range(BPC):
                n = c * BPC + k
                first, last = (n == 0), (n == TPB - 1)
                lhsT = mask[:, :, n]
                nc.tensor.matmul(ps0[:], lhsT, xt[:, k, 0:512], start=first, stop=last)
                nc.tensor.matmul(ps1[:], lhsT, xt[:, k, 512:E], start=first, stop=last)

        # ---- head-sum: [B, 1024] -> [B, 128] ----
        # (tensor_tensor may read at most one input from PSUM)
        sb1 = sp.tile([B, 512], F32)
        nc.vector.tensor_copy(sb1[:], ps1[:])
        s512 = sp.tile([B, 512], F32)
        nc.vector.tensor_tensor(s512[:], ps0[:], sb1[:], op=mybir.AluOpType.add)
        s256 = sp.tile([B, 256], F32)
        nc.vector.tensor_tensor(
            s256[:], s512[:, 0:256], s512[:, 256:512], op=mybir.AluOpType.add
        )
        pre = sp.tile([B, D], F32)
        nc.vector.tensor_tensor(
            pre[:], s256[:, 0:128], s256[:, 128:256], op=mybir.AluOpType.add
        )

        # ---- AllReduce partial sums across the 8 cores ----
        arin = dp.tile([B, D], F32)
        arout = dp.tile([B, D], F32)
        nc.sync.dma_start(arin[:], pre[:])
        nc.gpsimd.collective_compute(
            "AllReduce",
            mybir.AluOpType.add,
            replica_groups=[list(range(N_CORES))],
            ins=[arin.opt()],
            outs=[arout.opt()],
        )
        post = sp.tile([B, D], F32)
        nc.sync.dma_start(post[:], arout[:])

        # ---- pooled mean: sums / (H * max(count, 1)) ----
        pm = sp.tile([B, D], F32)
        nc.vector.tensor_scalar(
            pm[:], post[:], recip[:], None, op0=mybir.AluOpType.mult
        )

        # ---- transpose pooled mean -> a0 [128, 8] bf16 ----
        pmt = ppm.tile([D, B], F32, tag="mlp_ps")
        nc.tensor.transpose(pmt[:], pm[:], ident[:])
        a0 = sp.tile([D, B], BF16)
        nc.vector.tensor_copy(a0[:], pmt[:])

        # ---- MLP (activations kept transposed: [feature, batch]) ----
        a1 = _mlp_dense(nc, ppm, spa, ones_row, a0, w1_sb, b_sbs["b1"], D, 8 * D, True)
        a2 = _mlp_dense(nc, ppm, spa, ones_row, a1, w2_sb, b_sbs["b2"], 8 * D, 2 * D, False)
        a3 = _mlp_dense(nc, ppm, spa, ones_row, a2, w3_sb, b_sbs["b3"], 2 * D, 4 * D, True)
        a4 = _mlp_dense(nc, ppm, spa, ones_row, a3, w4_sb, b_sbs["b4"], 4 * D, D, True)
        a5 = _mlp_dense(nc, ppm, spa, ones_row, a4, w5_sb, b_sbs["b5"], D, 2, False)

        # ---- logits [2, 8] -> z[b] = (logit1 > logit0) -> out [8, 8] ----
        # a5 is bf16 [2, 8]; transpose needs f32-safe path: cast up via copy
        a5f = sp.tile([2, 8], F32)
        nc.vector.tensor_copy(a5f[:], a5[0:2, 0:8])
        lgt = ppm.tile([B, 2], F32, tag="mlp_ps")
        nc.tensor.transpose(lgt[:], a5f[:], ident[0:2, 0:2])
        lg = sp.tile([B, 2], F32)
        nc.vector.tensor_copy(lg[:], lgt[:])
        z = sp.tile([B, 1], F32)
        nc.vector.tensor_tensor(z[:], lg[:, 1:2], lg[:, 0:1], op=mybir.AluOpType.is_gt)
        zb = sp.tile([B, H], F32)
        nc.vector.tensor_scalar(zb[:], ones8[:], z[:], None, op0=mybir.AluOpType.mult)
        nc.sync.dma_start(d["out"].ap(), zb[:])


def build():
    nc = bacc.Bacc("TRN2", target_bir_lowering=False, debug=False, num_devices=N_CORES)
    d = {}
    d["x"] = nc.dram_tensor("x", [TOK, E], F32, kind="ExternalInput")
    d["tidx"] = nc.dram_tensor("tidx", [NPART, TPB], F32, kind="ExternalInput")
    d["cu"] = nc.dram_tensor("cu", [NPART, B + 1], F32, kind="ExternalInput")
    d["ident"] = nc.dram_tensor("ident", [8, 8], F32, kind="ExternalInput")
    d["w1"] = nc.dram_tensor("w1", [D, 8 * D], F32, kind="ExternalInput")
    d["b1"] = nc.dram_tensor("b1", [1, 8 * D], F32, kind="ExternalInput")
    d["w2"] = nc.dram_tensor("w2", [8 * D, 2 * D], F32, kind="ExternalInput")
    d["b2"] = nc.dram_tensor("b2", [1, 2 * D], F32, kind="ExternalInput")
    d["w3"] = nc.dram_tensor("w3", [2 * D, 4 * D], F32, kind="ExternalInput")
    d["b3"] = nc.dram_tensor("b3", [1, 4 * D], F32, kind="ExternalInput")
    d["w4"] = nc.dram_tensor("w4", [4 * D, D], F32, kind="ExternalInput")
    d["b4"] = nc.dram_tensor("b4", [1, D], F32, kind="ExternalInput")
    d["w5"] = nc.dram_tensor("w5", [D, 2], F32, kind="ExternalInput")
    d["b5"] = nc.dram_tensor("b5", [1, 2], F32, kind="ExternalInput")
    d["out"] = nc.dram_tensor("out", [B, H], F32, kind="ExternalOutput")
    with tile.TileContext(nc) as tc:
        _build_kernel_body(nc, tc, d)
    nc.compile()
    return nc


def make_in_maps(x, cu_seq_len, w1, b1, w2, b2, w3, b3, w4, b4, w5, b5):
    x = np.ascontiguousarray(np.asarray(x, dtype=np.float32)).reshape(T, E)
    cu_f = np.asarray(cu_seq_len, dtype=np.float32)
    cu_rep = np.ascontiguousarray(np.broadcast_to(cu_f, (NPART, B + 1)))
    ident = np.eye(8, dtype=np.float32)
    common = {
        "cu": cu_rep,
        "ident": ident,
        "w1": np.asarray(w1, np.float32), "b1": np.asarray(b1, np.float32).reshape(1, -1),
        "w2": np.asarray(w2, np.float32), "b2": np.asarray(b2, np.float32).reshape(1, -1),
        "w3": np.asarray(w3, np.float32), "b3": np.asarray(b3, np.float32).reshape(1, -1),
        "w4": np.asarray(w4, np.float32), "b4": np.asarray(b4, np.float32).reshape(1, -1),
        "w5": np.asarray(w5, np.float32), "b5": np.asarray(b5, np.float32).reshape(1, -1),
    }
    in_maps = []
    for c in range(N_CORES):
        tidx = (c * TOK + np.arange(TOK, dtype=np.float32)).reshape(NPART, TPB)
        in_maps.append({"x": x[c * TOK : (c + 1) * TOK], "tidx": tidx, **common})
    return in_maps


_NC_CACHE = {}


def _get_nc():
    if "nc" not in _NC_CACHE:
        _NC_CACHE["nc"] = build()
    return _NC_CACHE["nc"]


def kernel(**inputs):
    nc = _get_nc()
    in_maps = make_in_maps(**inputs)
    res = run_bass_kernel_spmd(nc, in_maps, core_ids=list(range(N_CORES)))
    out = np.asarray(res.results[0]["out"], dtype=np.float32)
    return out.reshape(B, H, 1)
